# revision 35
# baseline (speedup 1.0000x reference)
"""Single-head causal attention (B=4, S=4096, D=1024, H=64) on 8 trn2 cores.

Sharding: core c -> batch b = c % 4, role r = c // 4.
Per batch, the 8 global q-tiles (512 rows each) are interleaved:
role 0 owns global tiles {0,2,4,6}, role 1 owns {1,3,5,7}.

Uniform SPMD program (no branching; walrus allows at most one sync wait per
DMA, so everything per-core is data, not control flow):
- Each core loads only its OWN 2048 rows of x (8 MB), projects Q/K/V for
  them in bf16 (PE-transpose of x via identity matmuls, fp32 PSUM accum),
  then the batch pair exchanges projected K^T/V per 512-row chunk through
  pipelined AllGather collectives (128 KB each) into a role-major buffer.
- Local q-tile i (global tile g = 2i + r) runs a static schedule of 2i+2
  k-chunk slots.  Causality: the last two slots are multiplied by host-
  provided mask tiles (lower-triangle / all-ones / all-zeros by role); the
  all-zeros mask kills the beyond-diagonal chunk of even-role tiles in both
  the numerator and denominator (denominator = ones-column appended to V).
- Scores are computed transposed (sT[k,q]) so exp() output feeds the PV
  matmul directly; QK^T row-packs two 64-contraction matmuls in the PE
  array (K^T/Q^T duplicated to partitions 64:128 via identity matmuls).

Softmax skips the running-max: scores = Q.K/8 with |score| <~ 4 here, exp is
safe in fp32 and the reference's max-subtraction cancels exactly.
"""

import math

import ml_dtypes
import numpy as np

B, S, D, H = 4, 4096, 1024, 64
NT = 4          # local q-tiles per core (512 rows each)
QT = 512        # q-tile rows
KC = 512        # k-chunk size
NKB = 4         # 128-row k-blocks per chunk
NCHUNK = S // KC  # 8 global k-chunks

_compiled = None
TRACE = False
LAST_RESULT = None


def _build():
    import concourse.bass as bass
    import concourse.mybir as mybir
    from concourse import bacc
    from concourse.masks import make_identity
    from concourse.tile import TileContext

    fp32 = mybir.dt.float32
    bf16 = mybir.dt.bfloat16
    i32 = mybir.dt.int32
    AF = mybir.ActivationFunctionType

    nc = bacc.Bacc(None, target_bir_lowering=False)
    x_kv = nc.dram_tensor("x_kv", [NT * KC, D], fp32, kind="ExternalInput")
    wqk_d = nc.dram_tensor("wqk", [D, 128], bf16, kind="ExternalInput")
    wv_d = nc.dram_tensor("wv", [D, H], bf16, kind="ExternalInput")
    bqk_d = nc.dram_tensor("bqk", [128, 1], fp32, kind="ExternalInput")
    bv_d = nc.dram_tensor("bv", [128, H], fp32, kind="ExternalInput")
    maskA_d = nc.dram_tensor("maskA", [128, 2048], bf16, kind="ExternalInput")
    maskB_d = nc.dram_tensor("maskB", [128, 2048], bf16, kind="ExternalInput")
    y_d = nc.dram_tensor("y", [NT * QT, H], fp32, kind="ExternalOutput")
    NKVC = 64 * KC + 128 * NKB * H   # per-chunk K^T + compact V (bf16 elems)
    kv_out = nc.dram_tensor("kv_out", [NT, NKVC], bf16)
    kv_alls = [nc.dram_tensor(f"kv_all{c}", [2, NKVC], bf16) for c in range(NT)]

    with TileContext(nc) as tc:
        with (
            tc.tile_pool(name="const", bufs=1) as cpool,
            tc.tile_pool(name="stage", bufs=2) as spool,
            tc.tile_pool(name="xstage", bufs=8) as xpool,
            tc.tile_pool(name="pX", bufs=8) as ppool,
            tc.tile_pool(name="fin", bufs=2) as fpool,
            tc.tile_pool(name="psA", bufs=2, space="PSUM") as psA,   # misc
            tc.tile_pool(name="psS", bufs=2, space="PSUM") as psS,   # transposes+scores
            tc.tile_pool(name="psO", bufs=2, space="PSUM") as psO,   # out acc
        ):
            # ---------------- persistent SBUF ----------------
            wqk = cpool.tile([128, 8 * 128], bf16, tag="wqk")   # [d%128, (db,128)]
            wv = cpool.tile([128, 8 * H], bf16, tag="wv")
            bqk = cpool.tile([128, 1], fp32, tag="bqk")
            bv = cpool.tile([128, H], fp32, tag="bv")
            bqk_v = cpool.tile([128, 1], fp32, tag="bqkv")
            bv_v = cpool.tile([128, H], fp32, tag="bvv")
            maskA = cpool.tile([128, 2048], bf16, tag="maskA")
            maskB = cpool.tile([128, 2048], bf16, tag="maskB")
            id_bf = cpool.tile([128, 128], bf16, tag="idbf")
            id64 = cpool.tile([128, 64], bf16, tag="id64")
            id64a = cpool.tile([64, 64], bf16, tag="id64a")
            id_f32 = cpool.tile([128, 128], fp32, tag="idf32")
            KT = cpool.tile([128, S], bf16, tag="KT")       # rows 0:64 & 64:128 = K^T
            QTl = cpool.tile([128, NT * QT], bf16, tag="QTl")
            Vt = cpool.tile([128, NCHUNK * NKB * (H + 1)], bf16, tag="Vt")
            kt_own = cpool.tile([64, NT * KC], bf16, tag="ktown")
            v_own = cpool.tile([128, NT * NKB * H], bf16, tag="vown")

            # weights / biases / sched in
            nc.sync.dma_start(
                out=wqk.rearrange("p (db m) -> p db m", m=128),
                in_=wqk_d.rearrange("(db p) m -> p db m", p=128),
            )
            nc.sync.dma_start(
                out=wv.rearrange("p (db m) -> p db m", m=H),
                in_=wv_d.rearrange("(db p) m -> p db m", p=128),
            )
            nc.sync.dma_start(out=bqk[:], in_=bqk_d[:])
            nc.sync.dma_start(out=bv[:], in_=bv_d[:])
            nc.vector.tensor_copy(bqk_v[:], bqk[:])
            nc.vector.tensor_copy(bv_v[:], bv[:])
            nc.sync.dma_start(out=maskA[:], in_=maskA_d[:])
            nc.sync.dma_start(out=maskB[:], in_=maskB_d[:])

            make_identity(nc, id_bf[:])
            make_identity(nc, id64[64:128, :])
            make_identity(nc, id64a[:])
            make_identity(nc, id_f32[:])



            # ones column of V_aug (col 64 of every 65-group)
            v_grp = Vt.rearrange("p (n s) -> p n s", s=H + 1)
            nc.gpsimd.memset(v_grp[:, :, H:H + 1], 1.0)

            # ---- interleaved: project chunk-pair i, then attention tile i ----
            def load_x(c):
                x_sb = xpool.tile([128, 4 * D], bf16, tag="xs")   # 4 row-tiles
                nc.gpsimd.dma_start(
                    out=x_sb.rearrange("p (t d) -> p t d", d=D),
                    in_=x_kv[c * KC:(c + 1) * KC, :].rearrange("(t p) d -> p t d", p=128),
                )
                return x_sb

            def project_chunk(c, x_sb):
                xT = spool.tile([128, 8 * KC], bf16, tag="xT")    # (db, q)
                for db in range(8):
                    if db % 2 == 0:
                        tp_f = psS.tile([128, 2 * KC], fp32, tag="sT")
                        tp = tp_f.bitcast(bf16)[:, 0:512]
                    else:
                        tp = psA.tile([128, 512], bf16, tag="ps_misc")
                    for t in range(4):
                        nc.tensor.transpose(
                            tp[:, t * 128:(t + 1) * 128],
                            x_sb[:, t * D + db * 128: t * D + (db + 1) * 128], id_bf[:]
                        )
                    if db % 2 == 0:
                        nc.vector.tensor_copy(xT[:, db * KC:(db + 1) * KC], tp[:])
                    else:
                        nc.scalar.copy(xT[:, db * KC:(db + 1) * KC], tp[:])
                # QK projection (stacked: rows 0:64 Q^T, 64:128 K^T)
                ps_qk = psA.tile([128, KC], fp32, tag="ps_misc")
                for db in range(8):
                    nc.tensor.matmul(
                        ps_qk[:],
                        wqk[:, db * 128:(db + 1) * 128],
                        xT[:, db * KC:(db + 1) * KC],
                        start=(db == 0), stop=(db == 7),
                    )
                nc.vector.tensor_scalar_add(
                    QTl[0:64, c * KC:(c + 1) * KC], ps_qk[0:64, :], bqk_v[0:64, :]
                )
                nc.vector.tensor_scalar_add(
                    kt_own[:, c * KC:(c + 1) * KC], ps_qk[64:128, :], bqk_v[64:128, :]
                )
                # V projection (direct [k,h] layout), per 128-row block
                for kb in range(NKB):
                    ps_v = psA.tile([128, H], fp32, tag="ps_misc")
                    for db in range(8):
                        nc.tensor.matmul(
                            ps_v[:],
                            xT[:, db * KC + kb * 128: db * KC + (kb + 1) * 128],
                            wv[:, db * H:(db + 1) * H],
                            start=(db == 0), stop=(db == 7),
                        )
                    nc.vector.tensor_add(
                        v_own[:, (c * NKB + kb) * H:(c * NKB + kb + 1) * H],
                        ps_v[:], bv_v[:]
                    )

            def finish_q(i):
                sl = slice(i * QT, (i + 1) * QT)
                pu = psA.tile([128, KC], fp32, tag="ps_misc")
                nc.tensor.matmul(
                    pu[64:128, :], id64a[:], QTl[0:64, sl],
                    start=True, stop=True, tile_position=(0, 64),
                )
                nc.vector.tensor_copy(QTl[64:128, sl], pu[64:128, :])

            def kpos(j):
                # buffer position of global k-chunk j (role-major layout)
                return (j % 2) * NT + j // 2

            def attention_tile(i):
                nslot = 2 * i + 2
                oT = psO.tile([128, QT], fp32, tag="oT")
                for j in range(nslot):
                    jp = kpos(j)
                    pX = ppool.tile([128, NKB * KC], bf16, tag="pX")
                    for pr in range(2):
                        sT2 = psS.tile([128, 2 * KC], fp32, tag="sT")
                        for kk in range(2):
                            kb = 2 * pr + kk
                            half = 0 if kb % 2 == 0 else 64
                            nc.tensor.matmul(
                                sT2[:, kk * KC:(kk + 1) * KC],
                                KT[half:half + 64,
                                   jp * KC + kb * 128: jp * KC + (kb + 1) * 128],
                                QTl[half:half + 64, i * QT:(i + 1) * QT],
                                start=True, stop=True,
                            )
                        nc.scalar.activation(
                            pX[:, pr * 2 * KC:(pr + 1) * 2 * KC], sT2[:], AF.Exp,
                            scale=1.0 / math.sqrt(H),
                        )
                    if j >= nslot - 2:  # the two data-masked slots
                        mk = maskA if j == nslot - 2 else maskB
                        nc.vector.tensor_mul(pX[:], pX[:], mk[:])
                    for kb in range(NKB):
                        g = (jp * NKB + kb) * (H + 1)
                        nc.tensor.matmul(
                            oT[0:65, :],
                            Vt[:, g:g + H + 1],
                            pX[:, kb * KC:(kb + 1) * KC],
                            start=(j == 0 and kb == 0),
                            stop=(j == nslot - 1 and kb == NKB - 1),
                            skip_group_check=True,
                        )
                # finalize: transpose back, divide by denominator, store
                oT_sb = fpool.tile([128, QT], fp32, tag="oTsb")
                nc.vector.tensor_copy(oT_sb[0:65, :], oT[0:65, :])
                po = psA.tile([128, 4 * 65], fp32, tag="ps_misc")
                for t in range(NKB):
                    nc.tensor.transpose(
                        po[:, t * 65:(t + 1) * 65],
                        oT_sb[0:65, t * 128:(t + 1) * 128], id_f32[0:65, 0:65]
                    )
                rec = fpool.tile([128, 4], fp32, tag="rec")
                nc.vector.reciprocal(
                    rec[:], po.rearrange("p (t s) -> p t s", s=65)[:, :, 64:65]
                )
                y_sb = fpool.tile([128, NKB * H], fp32, tag="ysb")
                for t in range(NKB):
                    nc.vector.tensor_scalar_mul(
                        y_sb[:, t * H:(t + 1) * H], po[:, t * 65: t * 65 + H],
                        rec[:, t:t + 1]
                    )
                nc.sync.dma_start(
                    out=y_d[i * QT:(i + 1) * QT, :].rearrange("(t p) h -> p t h", p=128),
                    in_=y_sb.rearrange("p (t h) -> p t h", h=H),
                )

            KPART = 64 * KC   # K^T elems per chunk in the kv packet
            def exchange_chunk(c):
                # ship own chunk c, all-gather across the batch pair, unpack
                nc.sync.dma_start(
                    out=kv_out[c:c + 1, 0:KPART].rearrange("o (h s) -> (o h) s", s=KC),
                    in_=kt_own[:, c * KC:(c + 1) * KC],
                )
                nc.sync.dma_start(
                    out=kv_out[c:c + 1, KPART:].rearrange("o (k g) -> (o k) g", g=NKB * H),
                    in_=v_own[:, c * NKB * H:(c + 1) * NKB * H],
                )
                nc.gpsimd.collective_compute(
                    "AllGather",
                    mybir.AluOpType.bypass,
                    replica_groups=[[0, 4], [1, 5], [2, 6], [3, 7]],
                    ins=[kv_out[c:c + 1, :]],
                    outs=[kv_alls[c][:]],
                )
                for r in range(2):
                    bp = r * NT + c   # buffer position (role-major)
                    nc.sync.dma_start(
                        out=KT[64:128, bp * KC:(bp + 1) * KC],
                        in_=kv_alls[c][r, 0:KPART].rearrange("(h s) -> h s", s=KC),
                    )
                    vbase = bp * NKB * (H + 1)
                    nc.sync.dma_start(
                        out=Vt[:, vbase: vbase + NKB * (H + 1)]
                              .rearrange("k (n gg) -> k n gg", gg=H + 1)[:, :, 0:H],
                        in_=kv_alls[c][r, KPART:].rearrange("(k n gg) -> k n gg", n=NKB, gg=H),
                    )
                    pd = psA.tile([64, KC], fp32, tag="ps_misc")
                    nc.tensor.matmul(
                        pd[:], id64[64:128, :], KT[64:128, bp * KC:(bp + 1) * KC],
                        start=True, stop=True,
                    )
                    nc.vector.tensor_copy(KT[0:64, bp * KC:(bp + 1) * KC], pd[:])

            x_sbs = [load_x(c) for c in range(NT)]
            for c in range(NT):
                project_chunk(c, x_sbs[c])
                finish_q(c)
                exchange_chunk(c)
            for i in range(NT):
                attention_tile(i)

    nc.compile()
    return nc


def _masks_for(role: int):
    # tri[kb][p, f] = 1.0 where f >= kb*128 + p  (keep q >= k in diag chunk)
    p = np.arange(128)[:, None]
    f = np.arange(512)[None, :]
    tri = np.concatenate(
        [(f >= kb * 128 + p).astype(np.float32) for kb in range(NKB)], axis=1
    )
    ones = np.ones((128, 2048), dtype=np.float32)
    zero = np.zeros((128, 2048), dtype=np.float32)
    maskA = tri if role == 0 else ones
    maskB = zero if role == 0 else tri
    return (np.ascontiguousarray(maskA).astype(ml_dtypes.bfloat16),
            np.ascontiguousarray(maskB).astype(ml_dtypes.bfloat16))


def kernel(x, Wq_w, Wq_b, Wk_w, Wk_b, Wv_w, Wv_b):
    global _compiled
    from concourse.bass_utils import run_bass_kernel_spmd

    x = np.asarray(x, dtype=np.float32)
    wqk = np.concatenate([np.asarray(Wq_w), np.asarray(Wk_w)], axis=1).astype(ml_dtypes.bfloat16)
    bqk = np.concatenate([np.asarray(Wq_b), np.asarray(Wk_b)])[:, None].astype(np.float32)
    wv = np.asarray(Wv_w, dtype=np.float32).astype(ml_dtypes.bfloat16)
    bv = np.broadcast_to(np.asarray(Wv_b, dtype=np.float32)[None, :], (128, H)).copy()

    if _compiled is None:
        _compiled = _build()
    nc = _compiled

    in_maps = []
    for c in range(8):
        b, role = c % 4, c // 4
        mA, mB = _masks_for(role)
        x_own = np.ascontiguousarray(
            x[b].reshape(NCHUNK, KC, D)[role::2].reshape(NT * KC, D)
        )
        in_maps.append({
            "x_kv": x_own,
            "wqk": wqk, "wv": wv, "bqk": bqk, "bv": bv,
            "maskA": mA, "maskB": mB,
        })
    global LAST_RESULT
    kw = {}
    if TRACE:
        kw = dict(trace=True, trace_cores=list(range(8)))
    res = run_bass_kernel_spmd(nc, in_maps, core_ids=list(range(8)), **kw)
    LAST_RESULT = res

    out = np.empty((B, S, H), dtype=np.float32)
    for c in range(8):
        b, role = c % 4, c // 4
        y = res.results[c]["y"]
        for i in range(NT):
            g = 2 * i + role
            out[b, g * QT:(g + 1) * QT, :] = y[i * QT:(i + 1) * QT, :]
    return out


# revision 43
# speedup vs baseline: 1.7180x; 1.7180x over previous
"""Single-head causal attention (B=4, S=4096, D=1024, H=64) on 8 trn2 cores.

Sharding: core c -> batch b = c % 4, role r = c // 4.
Per batch, the 8 global q-tiles (512 rows each) are interleaved:
role 0 owns global tiles {0,2,4,6}, role 1 owns {1,3,5,7}.

Uniform SPMD program (no branching; walrus allows at most one sync wait per
DMA, so everything per-core is data, not control flow):
- Each core loads only its OWN 2048 rows of x (8 MB), projects Q/K/V for
  them in bf16 (PE-transpose of x via identity matmuls, fp32 PSUM accum),
  then the batch pair exchanges projected K^T/V per 512-row chunk through
  pipelined AllGather collectives (128 KB each) into a role-major buffer.
- Local q-tile i (global tile g = 2i + r) runs a static schedule of 2i+2
  k-chunk slots.  Causality: the last two slots are multiplied by host-
  provided mask tiles (lower-triangle / all-ones / all-zeros by role); the
  all-zeros mask kills the beyond-diagonal chunk of even-role tiles in both
  the numerator and denominator (denominator = ones-column appended to V).
- Scores are computed transposed (sT[k,q]) so exp() output feeds the PV
  matmul directly; QK^T row-packs two 64-contraction matmuls in the PE
  array (K^T/Q^T duplicated to partitions 64:128 via identity matmuls).

Softmax skips the running-max: scores = Q.K/8 with |score| <~ 4 here, exp is
safe in fp32 and the reference's max-subtraction cancels exactly.
"""

import math

import ml_dtypes
import numpy as np

B, S, D, H = 4, 4096, 1024, 64
NT = 4          # local q-tiles per core (512 rows each)
QT = 512        # q-tile rows
KC = 512        # k-chunk size
NKB = 4         # 128-row k-blocks per chunk
NCHUNK = S // KC  # 8 global k-chunks

_compiled = None
TRACE = False
LAST_RESULT = None


def _build():
    import concourse.bass as bass
    import concourse.mybir as mybir
    from concourse import bacc
    from concourse.masks import make_identity
    from concourse.tile import TileContext

    fp32 = mybir.dt.float32
    bf16 = mybir.dt.bfloat16
    i32 = mybir.dt.int32
    AF = mybir.ActivationFunctionType

    nc = bacc.Bacc(None, target_bir_lowering=False)
    x_kv = nc.dram_tensor("x_kv", [NT * KC, D], fp32, kind="ExternalInput")
    wqk_d = nc.dram_tensor("wqk", [128, 8 * 128], bf16, kind="ExternalInput")
    wv_d = nc.dram_tensor("wv", [128, 8 * H], bf16, kind="ExternalInput")
    bqk_d = nc.dram_tensor("bqk", [128, 1], fp32, kind="ExternalInput")
    bv_d = nc.dram_tensor("bv", [128, H], fp32, kind="ExternalInput")
    maskA_d = nc.dram_tensor("maskA", [128, 2048], bf16, kind="ExternalInput")
    maskB_d = nc.dram_tensor("maskB", [128, 2048], bf16, kind="ExternalInput")
    y_d = nc.dram_tensor("y", [NT * QT, H], fp32, kind="ExternalOutput")
    NKVC = 64 * KC + 128 * NKB * H   # per-chunk K^T + compact V (bf16 elems)
    kv_out = nc.dram_tensor("kv_out", [NT, NKVC], bf16)
    kv_alls = [nc.dram_tensor(f"kv_all{c}", [2, NKVC], bf16) for c in range(NT)]

    with TileContext(nc) as tc:
        with (
            tc.tile_pool(name="const", bufs=1) as cpool,
            tc.tile_pool(name="stage", bufs=3) as spool,
            tc.tile_pool(name="xstage", bufs=8) as xpool,
            tc.tile_pool(name="pX", bufs=8) as ppool,
            tc.tile_pool(name="fin", bufs=2) as fpool,
            tc.tile_pool(name="psA", bufs=2, space="PSUM") as psA,   # misc
            tc.tile_pool(name="psS", bufs=2, space="PSUM") as psS,   # transposes+scores
            tc.tile_pool(name="psO", bufs=2, space="PSUM") as psO,   # out acc
        ):
            # ---------------- persistent SBUF ----------------
            wqk = cpool.tile([128, 8 * 128], bf16, tag="wqk")   # [d%128, (db,128)]
            wv = cpool.tile([128, 8 * H], bf16, tag="wv")
            bqk = cpool.tile([128, 1], fp32, tag="bqk")
            bv = cpool.tile([128, H], fp32, tag="bv")
            bqk_v = cpool.tile([128, 1], fp32, tag="bqkv")
            bv_v = cpool.tile([128, H], fp32, tag="bvv")
            maskA = cpool.tile([128, 2048], bf16, tag="maskA")
            maskB = cpool.tile([128, 2048], bf16, tag="maskB")
            id_bf = cpool.tile([128, 128], bf16, tag="idbf")
            id64 = cpool.tile([128, 64], bf16, tag="id64")
            id64a = cpool.tile([64, 64], bf16, tag="id64a")
            id_f32 = cpool.tile([128, 128], fp32, tag="idf32")
            KT = cpool.tile([128, S], bf16, tag="KT")       # rows 0:64 & 64:128 = K^T
            QTl = cpool.tile([128, NT * QT], bf16, tag="QTl")
            Vt = cpool.tile([128, NCHUNK * NKB * (H + 1)], bf16, tag="Vt")
            kt_own = cpool.tile([64, NT * KC], bf16, tag="ktown")
            v_own = cpool.tile([128, NT * NKB * H], bf16, tag="vown")

            # weights / biases / sched in
            nc.sync.dma_start(out=wqk[:], in_=wqk_d[:])
            nc.sync.dma_start(out=wv[:], in_=wv_d[:])
            nc.sync.dma_start(out=bqk[:], in_=bqk_d[:])
            nc.sync.dma_start(out=bv[:], in_=bv_d[:])
            nc.vector.tensor_copy(bqk_v[:], bqk[:])
            nc.vector.tensor_copy(bv_v[:], bv[:])
            nc.scalar.dma_start(out=maskA[:], in_=maskA_d[:])
            nc.scalar.dma_start(out=maskB[:], in_=maskB_d[:])

            make_identity(nc, id_bf[:])
            make_identity(nc, id64[64:128, :])
            make_identity(nc, id64a[:])
            make_identity(nc, id_f32[:])



            # ones column of V_aug (col 64 of every 65-group)
            v_grp = Vt.rearrange("p (n s) -> p n s", s=H + 1)
            nc.gpsimd.memset(v_grp[:, :, H:H + 1], 1.0)

            KPART = 64 * KC   # K^T elems per chunk in the kv packet

            # ---- interleaved: project chunk-pair i, then attention tile i ----
            def load_x(c):
                x_sb = xpool.tile([128, 4 * D], bf16, tag="xs")
                for t in range(4):
                    nc.gpsimd.dma_start(
                        out=x_sb[:, t * D:(t + 1) * D],
                        in_=x_kv[c * KC + t * 128: c * KC + (t + 1) * 128, :],
                    )
                return x_sb

            def project_chunk(c, x_sb):
                xT = spool.tile([128, 8 * KC], bf16, tag="xT")    # (db, q)
                for db in range(8):
                    if db % 2 == 0:
                        tp_f = psS.tile([128, 2 * KC], fp32, tag="sT")
                        tp = tp_f.bitcast(bf16)[:, 0:512]
                    else:
                        tp = psA.tile([128, 512], bf16, tag="ps_misc")
                    for t in range(4):
                        nc.tensor.transpose(
                            tp[:, t * 128:(t + 1) * 128],
                            x_sb[:, t * D + db * 128: t * D + (db + 1) * 128], id_bf[:]
                        )
                    nc.vector.tensor_copy(xT[:, db * KC:(db + 1) * KC], tp[:])
                # QK projection (stacked: rows 0:64 Q^T, 64:128 K^T)
                ps_qk = psA.tile([128, KC], fp32, tag="ps_misc")
                for db in range(8):
                    nc.tensor.matmul(
                        ps_qk[:],
                        wqk[:, db * 128:(db + 1) * 128],
                        xT[:, db * KC:(db + 1) * KC],
                        start=(db == 0), stop=(db == 7),
                    )
                nc.vector.tensor_scalar_add(
                    QTl[0:64, c * KC:(c + 1) * KC], ps_qk[0:64, :], bqk_v[0:64, :]
                )
                nc.vector.tensor_scalar_add(
                    kt_own[:, c * KC:(c + 1) * KC], ps_qk[64:128, :], bqk_v[64:128, :]
                )
                # V projection (direct [k,h] layout), per 128-row block
                for kb in range(NKB):
                    ps_v = psA.tile([128, H], fp32, tag="ps_misc")
                    for db in range(8):
                        nc.tensor.matmul(
                            ps_v[:],
                            xT[:, db * KC + kb * 128: db * KC + (kb + 1) * 128],
                            wv[:, db * H:(db + 1) * H],
                            start=(db == 0), stop=(db == 7),
                        )
                    nc.vector.tensor_add(
                        v_own[:, (c * NKB + kb) * H:(c * NKB + kb + 1) * H],
                        ps_v[:], bv_v[:]
                    )

            def finish_q(i):
                sl = slice(i * QT, (i + 1) * QT)
                pu = psA.tile([128, KC], fp32, tag="ps_misc")
                nc.tensor.matmul(
                    pu[64:128, :], id64a[:], QTl[0:64, sl],
                    start=True, stop=True, tile_position=(0, 64),
                )
                nc.vector.tensor_copy(QTl[64:128, sl], pu[64:128, :])

            def kpos(j):
                # buffer position of global k-chunk j (role-major layout)
                return (j % 2) * NT + j // 2

            def attention_tile(i):
                nslot = 2 * i + 2
                oT = psO.tile([128, QT], fp32, tag="oT")
                for j in range(nslot):
                    jp = kpos(j)
                    pX = ppool.tile([128, NKB * KC], bf16, tag="pX")
                    for pr in range(2):
                        sT2 = psS.tile([128, 2 * KC], fp32, tag="sT")
                        for kk in range(2):
                            kb = 2 * pr + kk
                            half = 0 if kb % 2 == 0 else 64
                            nc.tensor.matmul(
                                sT2[:, kk * KC:(kk + 1) * KC],
                                KT[half:half + 64,
                                   jp * KC + kb * 128: jp * KC + (kb + 1) * 128],
                                QTl[half:half + 64, i * QT:(i + 1) * QT],
                                start=True, stop=True,
                            )
                        nc.scalar.activation(
                            pX[:, pr * 2 * KC:(pr + 1) * 2 * KC], sT2[:], AF.Exp,
                            scale=1.0 / math.sqrt(H),
                        )
                    if j >= nslot - 2:  # the two data-masked slots
                        mk = maskA if j == nslot - 2 else maskB
                        nc.vector.tensor_mul(pX[:], pX[:], mk[:])
                    for kb in range(NKB):
                        g = (jp * NKB + kb) * (H + 1)
                        nc.tensor.matmul(
                            oT[0:65, :],
                            Vt[:, g:g + H + 1],
                            pX[:, kb * KC:(kb + 1) * KC],
                            start=(j == 0 and kb == 0),
                            stop=(j == nslot - 1 and kb == NKB - 1),
                            skip_group_check=True,
                        )
                # finalize: transpose back, divide by denominator, store
                oT_sb = fpool.tile([128, QT], fp32, tag="oTsb")
                nc.vector.tensor_copy(oT_sb[0:65, :], oT[0:65, :])
                po = psA.tile([128, 4 * 65], fp32, tag="ps_misc")
                for t in range(NKB):
                    nc.tensor.transpose(
                        po[:, t * 65:(t + 1) * 65],
                        oT_sb[0:65, t * 128:(t + 1) * 128], id_f32[0:65, 0:65]
                    )
                rec = fpool.tile([128, 4], fp32, tag="rec")
                nc.vector.reciprocal(
                    rec[:], po.rearrange("p (t s) -> p t s", s=65)[:, :, 64:65]
                )
                y_sb = fpool.tile([128, NKB * H], fp32, tag="ysb")
                for t in range(NKB):
                    nc.vector.tensor_scalar_mul(
                        y_sb[:, t * H:(t + 1) * H], po[:, t * 65: t * 65 + H],
                        rec[:, t:t + 1]
                    )
                nc.sync.dma_start(
                    out=y_d[i * QT:(i + 1) * QT, :].rearrange("(t p) h -> p t h", p=128),
                    in_=y_sb.rearrange("p (t h) -> p t h", h=H),
                )

            def exchange_chunk(c):
                # ship own K/V of chunk c, all-gather across the batch pair, unpack
                nc.sync.dma_start(
                    out=kv_out[c:c + 1, 0:KPART].rearrange("o (h s) -> (o h) s", s=KC),
                    in_=kt_own[:, c * KC:(c + 1) * KC],
                )
                nc.sync.dma_start(
                    out=kv_out[c:c + 1, KPART:].rearrange("o (k g) -> (o k) g", g=NKB * H),
                    in_=v_own[:, c * NKB * H:(c + 1) * NKB * H],
                )
                nc.gpsimd.collective_compute(
                    "AllGather",
                    mybir.AluOpType.bypass,
                    replica_groups=[[0, 4], [1, 5], [2, 6], [3, 7]],
                    ins=[kv_out[c:c + 1, :]],
                    outs=[kv_alls[c][:]],
                )
                for r in range(2):
                    bp = r * NT + c   # buffer position (role-major)
                    nc.scalar.dma_start(
                        out=KT[64:128, bp * KC:(bp + 1) * KC],
                        in_=kv_alls[c][r, 0:KPART].rearrange("(h s) -> h s", s=KC),
                    )
                    nc.scalar.dma_start(
                        out=KT[0:64, bp * KC:(bp + 1) * KC],
                        in_=kv_alls[c][r, 0:KPART].rearrange("(h s) -> h s", s=KC),
                    )
                    vbase = bp * NKB * (H + 1)
                    nc.scalar.dma_start(
                        out=Vt[:, vbase: vbase + NKB * (H + 1)]
                              .rearrange("k (n gg) -> k n gg", gg=H + 1)[:, :, 0:H],
                        in_=kv_alls[c][r, KPART:].rearrange("(k n gg) -> k n gg", n=NKB, gg=H),
                    )

            x_sbs = [load_x(c) for c in range(NT)]
            for c in range(NT):
                project_chunk(c, x_sbs[c])
                finish_q(c)
                exchange_chunk(c)
            for i in range(NT):
                attention_tile(i)

    nc.compile()
    return nc


def _masks_for(role: int):
    # tri[kb][p, f] = 1.0 where f >= kb*128 + p  (keep q >= k in diag chunk)
    p = np.arange(128)[:, None]
    f = np.arange(512)[None, :]
    tri = np.concatenate(
        [(f >= kb * 128 + p).astype(np.float32) for kb in range(NKB)], axis=1
    )
    ones = np.ones((128, 2048), dtype=np.float32)
    zero = np.zeros((128, 2048), dtype=np.float32)
    maskA = tri if role == 0 else ones
    maskB = zero if role == 0 else tri
    return (np.ascontiguousarray(maskA).astype(ml_dtypes.bfloat16),
            np.ascontiguousarray(maskB).astype(ml_dtypes.bfloat16))


def kernel(x, Wq_w, Wq_b, Wk_w, Wk_b, Wv_w, Wv_b):
    global _compiled
    from concourse.bass_utils import run_bass_kernel_spmd

    x = np.asarray(x, dtype=np.float32)
    wqk_dm = np.concatenate([np.asarray(Wq_w), np.asarray(Wk_w)], axis=1)
    wqk = np.ascontiguousarray(
        wqk_dm.reshape(8, 128, 128).transpose(1, 0, 2).reshape(128, 8 * 128)
    ).astype(ml_dtypes.bfloat16)
    bqk = np.concatenate([np.asarray(Wq_b), np.asarray(Wk_b)])[:, None].astype(np.float32)
    wv = np.ascontiguousarray(
        np.asarray(Wv_w, dtype=np.float32).reshape(8, 128, H)
        .transpose(1, 0, 2).reshape(128, 8 * H)
    ).astype(ml_dtypes.bfloat16)
    bv = np.broadcast_to(np.asarray(Wv_b, dtype=np.float32)[None, :], (128, H)).copy()

    if _compiled is None:
        _compiled = _build()
    nc = _compiled

    in_maps = []
    for c in range(8):
        b, role = c % 4, c // 4
        mA, mB = _masks_for(role)
        x_own = np.ascontiguousarray(
            x[b].reshape(NCHUNK, KC, D)[role::2].reshape(NT * KC, D)
        )
        in_maps.append({
            "x_kv": x_own,
            "wqk": wqk, "wv": wv, "bqk": bqk, "bv": bv,
            "maskA": mA, "maskB": mB,
        })
    global LAST_RESULT
    kw = {}
    if TRACE:
        kw = dict(trace=True, trace_cores=list(range(8)))
    res = run_bass_kernel_spmd(nc, in_maps, core_ids=list(range(8)), **kw)
    LAST_RESULT = res

    out = np.empty((B, S, H), dtype=np.float32)
    for c in range(8):
        b, role = c % 4, c // 4
        y = res.results[c]["y"]
        for i in range(NT):
            g = 2 * i + role
            out[b, g * QT:(g + 1) * QT, :] = y[i * QT:(i + 1) * QT, :]
    return out


# revision 53
# speedup vs baseline: 1.7411x; 1.0135x over previous
"""Single-head causal attention (B=4, S=4096, D=1024, H=64) on 8 trn2 cores.

Sharding: core c -> batch b = c % 4, role r = c // 4.
Per batch, the 8 global q-tiles (512 rows each) are interleaved:
role 0 owns global tiles {0,2,4,6}, role 1 owns {1,3,5,7}.

Uniform SPMD program (no branching; walrus allows at most one sync wait per
DMA, so everything per-core is data, not control flow):
- Each core loads only its OWN 2048 rows of x (8 MB), projects Q/K/V for
  them in bf16 (PE-transpose of x via identity matmuls, fp32 PSUM accum),
  then the batch pair exchanges projected K^T/V per 512-row chunk through
  pipelined AllGather collectives (128 KB each) into a role-major buffer.
- Local q-tile i (global tile g = 2i + r) runs a static schedule of 2i+2
  k-chunk slots.  Causality: the last two slots are multiplied by host-
  provided mask tiles (lower-triangle / all-ones / all-zeros by role); the
  all-zeros mask kills the beyond-diagonal chunk of even-role tiles in both
  the numerator and denominator (denominator = ones-column appended to V).
- Scores are computed transposed (sT[k,q]) so exp() output feeds the PV
  matmul directly; QK^T row-packs two 64-contraction matmuls in the PE
  array (K^T/Q^T duplicated to partitions 64:128 via identity matmuls).

Softmax skips the running-max: scores = Q.K/8 with |score| <~ 4 here, exp is
safe in fp32 and the reference's max-subtraction cancels exactly.
"""

import math

import ml_dtypes
import numpy as np

B, S, D, H = 4, 4096, 1024, 64
NT = 4          # local q-tiles per core (512 rows each)
QT = 512        # q-tile rows
KC = 512        # k-chunk size
NKB = 4         # 128-row k-blocks per chunk
NCHUNK = S // KC  # 8 global k-chunks

_compiled = None
TRACE = False
LAST_RESULT = None


def _build():
    import concourse.bass as bass
    import concourse.mybir as mybir
    from concourse import bacc
    from concourse.masks import make_identity
    from concourse.tile import TileContext

    fp32 = mybir.dt.float32
    bf16 = mybir.dt.bfloat16
    i32 = mybir.dt.int32
    AF = mybir.ActivationFunctionType

    nc = bacc.Bacc(None, target_bir_lowering=False)
    x_kv = nc.dram_tensor("x_kv", [NT * KC, D], fp32, kind="ExternalInput")
    wqk_d = nc.dram_tensor("wqk", [128, 8 * 128], bf16, kind="ExternalInput")
    wv_d = nc.dram_tensor("wv", [128, 8 * H], bf16, kind="ExternalInput")
    bqk_d = nc.dram_tensor("bqk", [128, 1], fp32, kind="ExternalInput")
    bv_d = nc.dram_tensor("bv", [128, H], fp32, kind="ExternalInput")
    maskA_d = nc.dram_tensor("maskA", [128, 2048], bf16, kind="ExternalInput")
    maskB_d = nc.dram_tensor("maskB", [128, 2048], bf16, kind="ExternalInput")
    y_d = nc.dram_tensor("y", [NT * QT, H], fp32, kind="ExternalOutput")
    NKVC = 64 * KC + 128 * NKB * H   # per-chunk K^T + compact V (bf16 elems)
    kv_out = nc.dram_tensor("kv_out", [NT, NKVC], bf16)
    kv_alls = [nc.dram_tensor(f"kv_all{c}", [2, NKVC], bf16) for c in range(NT)]

    with TileContext(nc) as tc:
        with (
            tc.tile_pool(name="const", bufs=1) as cpool,
            tc.tile_pool(name="stage", bufs=3) as spool,
            tc.tile_pool(name="xstage", bufs=8) as xpool,
            tc.tile_pool(name="pX", bufs=16) as ppool,
            tc.tile_pool(name="fin", bufs=2) as fpool,
            tc.tile_pool(name="psA", bufs=2, space="PSUM") as psA,   # misc
            tc.tile_pool(name="psS", bufs=2, space="PSUM") as psS,   # transposes+scores
            tc.tile_pool(name="psO", bufs=2, space="PSUM") as psO,   # out acc
        ):
            # ---------------- persistent SBUF ----------------
            wqk = cpool.tile([128, 8 * 128], bf16, tag="wqk")   # [d%128, (db,128)]
            wv = cpool.tile([128, 8 * H], bf16, tag="wv")
            bqk = cpool.tile([128, 1], fp32, tag="bqk")
            bv = cpool.tile([128, H], fp32, tag="bv")
            bqk_v = cpool.tile([128, 1], fp32, tag="bqkv")
            bv_v = cpool.tile([128, H], fp32, tag="bvv")
            maskA = cpool.tile([128, 2048], bf16, tag="maskA")
            maskB = cpool.tile([128, 2048], bf16, tag="maskB")
            id_bf = cpool.tile([128, 128], bf16, tag="idbf")
            id64 = cpool.tile([128, 64], bf16, tag="id64")
            id64a = cpool.tile([64, 64], bf16, tag="id64a")
            id_f32 = cpool.tile([128, 128], fp32, tag="idf32")
            KT = cpool.tile([128, S], bf16, tag="KT")       # rows 0:64 & 64:128 = K^T
            QTl = cpool.tile([128, NT * QT], bf16, tag="QTl")
            Vt = cpool.tile([128, NCHUNK * NKB * (H + 1)], bf16, tag="Vt")
            kt_own = cpool.tile([64, NT * KC], bf16, tag="ktown")
            v_own = cpool.tile([128, NT * NKB * H], bf16, tag="vown")

            # weights / biases / sched in
            nc.sync.dma_start(out=wqk[:], in_=wqk_d[:])
            nc.sync.dma_start(out=wv[:], in_=wv_d[:])
            nc.sync.dma_start(out=bqk[:], in_=bqk_d[:])
            nc.sync.dma_start(out=bv[:], in_=bv_d[:])
            nc.vector.tensor_copy(bqk_v[:], bqk[:])
            nc.vector.tensor_copy(bv_v[:], bv[:])
            nc.scalar.dma_start(out=maskA[:], in_=maskA_d[:])
            nc.scalar.dma_start(out=maskB[:], in_=maskB_d[:])

            make_identity(nc, id_bf[:])
            make_identity(nc, id64[64:128, :])
            make_identity(nc, id64a[:])
            make_identity(nc, id_f32[:])



            # ones column of V_aug (col 64 of every 65-group)
            v_grp = Vt.rearrange("p (n s) -> p n s", s=H + 1)
            nc.vector.memset(v_grp[:, :, H:H + 1], 1.0)

            KPART = 64 * KC   # K^T elems per chunk in the kv packet

            # ---- interleaved: project chunk-pair i, then attention tile i ----
            def load_x(c):
                pieces = []
                for hh in range(2):
                    xp = xpool.tile([128, 2 * D], bf16, tag="xs")
                    nc.gpsimd.dma_start(
                        out=xp.rearrange("p (t d) -> p t d", d=D),
                        in_=x_kv[c * KC + hh * 256: c * KC + (hh + 1) * 256, :]
                              .rearrange("(t p) d -> p t d", p=128),
                    )
                    pieces.append(xp)
                return pieces

            def project_chunk(c, x_pieces):
                xT = spool.tile([128, 8 * KC], bf16, tag="xT")    # (db, q)
                for db in range(8):
                    if db % 2 == 0:
                        tp_f = psS.tile([128, 2 * KC], fp32, tag="sT")
                        tp = tp_f.bitcast(bf16)[:, 0:512]
                    else:
                        tp = psA.tile([128, 512], bf16, tag="ps_misc")
                    for t in range(4):
                        nc.tensor.transpose(
                            tp[:, t * 128:(t + 1) * 128],
                            x_pieces[t // 2][:, (t % 2) * D + db * 128:
                                             (t % 2) * D + (db + 1) * 128], id_bf[:]
                        )
                    nc.vector.tensor_copy(xT[:, db * KC:(db + 1) * KC], tp[:])
                # QK projection (stacked: rows 0:64 Q^T, 64:128 K^T)
                ps_qk = psA.tile([128, KC], fp32, tag="ps_misc")
                for db in range(8):
                    nc.tensor.matmul(
                        ps_qk[:],
                        wqk[:, db * 128:(db + 1) * 128],
                        xT[:, db * KC:(db + 1) * KC],
                        start=(db == 0), stop=(db == 7),
                    )
                nc.vector.tensor_scalar_add(
                    QTl[0:64, c * KC:(c + 1) * KC], ps_qk[0:64, :], bqk_v[0:64, :]
                )
                nc.vector.tensor_scalar_add(
                    kt_own[:, c * KC:(c + 1) * KC], ps_qk[64:128, :], bqk_v[64:128, :]
                )
                # V projection (direct [k,h] layout), per 128-row block
                for kb in range(NKB):
                    ps_v = psA.tile([128, H], fp32, tag="ps_misc")
                    for db in range(8):
                        nc.tensor.matmul(
                            ps_v[:],
                            xT[:, db * KC + kb * 128: db * KC + (kb + 1) * 128],
                            wv[:, db * H:(db + 1) * H],
                            start=(db == 0), stop=(db == 7),
                        )
                    nc.vector.tensor_add(
                        v_own[:, (c * NKB + kb) * H:(c * NKB + kb + 1) * H],
                        ps_v[:], bv_v[:]
                    )

            def finish_q(i):
                sl = slice(i * QT, (i + 1) * QT)
                pu = psA.tile([128, KC], fp32, tag="ps_misc")
                nc.tensor.matmul(
                    pu[64:128, :], id64a[:], QTl[0:64, sl],
                    start=True, stop=True, tile_position=(0, 64),
                )
                nc.vector.tensor_copy(QTl[64:128, sl], pu[64:128, :])

            def kpos(j):
                # buffer position of global k-chunk j (role-major layout)
                return (j % 2) * NT + j // 2

            def attention_tile(i):
                nslot = 2 * i + 2
                oT = psO.tile([128, QT], fp32, tag="oT")
                for j in range(nslot):
                    jp = kpos(j)
                    pX = ppool.tile([128, NKB * KC], bf16, tag="pX")
                    for pr in range(2):
                        sT2 = psS.tile([128, 2 * KC], fp32, tag="sT")
                        for kk in range(2):
                            kb = 2 * pr + kk
                            half = 0 if kb % 2 == 0 else 64
                            nc.tensor.matmul(
                                sT2[:, kk * KC:(kk + 1) * KC],
                                KT[half:half + 64,
                                   jp * KC + kb * 128: jp * KC + (kb + 1) * 128],
                                QTl[half:half + 64, i * QT:(i + 1) * QT],
                                start=True, stop=True,
                            )
                        nc.scalar.activation(
                            pX[:, pr * 2 * KC:(pr + 1) * 2 * KC], sT2[:], AF.Exp,
                            scale=1.0 / math.sqrt(H),
                        )
                    if j >= nslot - 2:  # the two data-masked slots
                        mk = maskA if j == nslot - 2 else maskB
                        nc.vector.tensor_mul(pX[:], pX[:], mk[:])
                    for kb in range(NKB):
                        g = (jp * NKB + kb) * (H + 1)
                        nc.tensor.matmul(
                            oT[0:65, :],
                            Vt[:, g:g + H + 1],
                            pX[:, kb * KC:(kb + 1) * KC],
                            start=(j == 0 and kb == 0),
                            stop=(j == nslot - 1 and kb == NKB - 1),
                            skip_group_check=True,
                        )
                # finalize: transpose back, divide by denominator, store
                oT_sb = fpool.tile([128, QT], fp32, tag="oTsb")
                nc.vector.tensor_copy(oT_sb[0:65, :], oT[0:65, :])
                po = psA.tile([128, 4 * 65], fp32, tag="ps_misc")
                for t in range(NKB):
                    nc.tensor.transpose(
                        po[:, t * 65:(t + 1) * 65],
                        oT_sb[0:65, t * 128:(t + 1) * 128], id_f32[0:65, 0:65]
                    )
                rec = fpool.tile([128, 4], fp32, tag="rec")
                nc.vector.reciprocal(
                    rec[:], po.rearrange("p (t s) -> p t s", s=65)[:, :, 64:65]
                )
                y_sb = fpool.tile([128, NKB * H], fp32, tag="ysb")
                for t in range(NKB):
                    nc.vector.tensor_scalar_mul(
                        y_sb[:, t * H:(t + 1) * H], po[:, t * 65: t * 65 + H],
                        rec[:, t:t + 1]
                    )
                nc.sync.dma_start(
                    out=y_d[i * QT:(i + 1) * QT, :].rearrange("(t p) h -> p t h", p=128),
                    in_=y_sb.rearrange("p (t h) -> p t h", h=H),
                )

            def exchange_chunk(c):
                # ship own K/V of chunk c, all-gather across the batch pair, unpack
                nc.sync.dma_start(
                    out=kv_out[c:c + 1, 0:KPART].rearrange("o (h s) -> (o h) s", s=KC),
                    in_=kt_own[:, c * KC:(c + 1) * KC],
                )
                nc.sync.dma_start(
                    out=kv_out[c:c + 1, KPART:].rearrange("o (k g) -> (o k) g", g=NKB * H),
                    in_=v_own[:, c * NKB * H:(c + 1) * NKB * H],
                )
                nc.gpsimd.collective_compute(
                    "AllGather",
                    mybir.AluOpType.bypass,
                    replica_groups=[[0, 4], [1, 5], [2, 6], [3, 7]],
                    ins=[kv_out[c:c + 1, :]],
                    outs=[kv_alls[c][:]],
                )
                for r in range(2):
                    bp = r * NT + c   # buffer position (role-major)
                    nc.scalar.dma_start(
                        out=KT[64:128, bp * KC:(bp + 1) * KC],
                        in_=kv_alls[c][r, 0:KPART].rearrange("(h s) -> h s", s=KC),
                    )
                    nc.scalar.dma_start(
                        out=KT[0:64, bp * KC:(bp + 1) * KC],
                        in_=kv_alls[c][r, 0:KPART].rearrange("(h s) -> h s", s=KC),
                    )
                    vbase = bp * NKB * (H + 1)
                    nc.scalar.dma_start(
                        out=Vt[:, vbase: vbase + NKB * (H + 1)]
                              .rearrange("k (n gg) -> k n gg", gg=H + 1)[:, :, 0:H],
                        in_=kv_alls[c][r, KPART:].rearrange("(k n gg) -> k n gg", n=NKB, gg=H),
                    )

            x_sbs = [load_x(c) for c in range(NT)]
            for c in range(NT):
                project_chunk(c, x_sbs[c])
                finish_q(c)
                exchange_chunk(c)
                attention_tile(c)

    nc.compile()
    return nc


def _masks_for(role: int):
    # tri[kb][p, f] = 1.0 where f >= kb*128 + p  (keep q >= k in diag chunk)
    p = np.arange(128)[:, None]
    f = np.arange(512)[None, :]
    tri = np.concatenate(
        [(f >= kb * 128 + p).astype(np.float32) for kb in range(NKB)], axis=1
    )
    ones = np.ones((128, 2048), dtype=np.float32)
    zero = np.zeros((128, 2048), dtype=np.float32)
    maskA = tri if role == 0 else ones
    maskB = zero if role == 0 else tri
    return (np.ascontiguousarray(maskA).astype(ml_dtypes.bfloat16),
            np.ascontiguousarray(maskB).astype(ml_dtypes.bfloat16))


def kernel(x, Wq_w, Wq_b, Wk_w, Wk_b, Wv_w, Wv_b):
    global _compiled
    from concourse.bass_utils import run_bass_kernel_spmd

    x = np.asarray(x, dtype=np.float32)
    wqk_dm = np.concatenate([np.asarray(Wq_w), np.asarray(Wk_w)], axis=1)
    wqk = np.ascontiguousarray(
        wqk_dm.reshape(8, 128, 128).transpose(1, 0, 2).reshape(128, 8 * 128)
    ).astype(ml_dtypes.bfloat16)
    bqk = np.concatenate([np.asarray(Wq_b), np.asarray(Wk_b)])[:, None].astype(np.float32)
    wv = np.ascontiguousarray(
        np.asarray(Wv_w, dtype=np.float32).reshape(8, 128, H)
        .transpose(1, 0, 2).reshape(128, 8 * H)
    ).astype(ml_dtypes.bfloat16)
    bv = np.broadcast_to(np.asarray(Wv_b, dtype=np.float32)[None, :], (128, H)).copy()

    if _compiled is None:
        _compiled = _build()
    nc = _compiled

    in_maps = []
    for c in range(8):
        b, role = c % 4, c // 4
        mA, mB = _masks_for(role)
        x_own = np.ascontiguousarray(
            x[b].reshape(NCHUNK, KC, D)[role::2].reshape(NT * KC, D)
        )
        in_maps.append({
            "x_kv": x_own,
            "wqk": wqk, "wv": wv, "bqk": bqk, "bv": bv,
            "maskA": mA, "maskB": mB,
        })
    global LAST_RESULT
    kw = {}
    if TRACE:
        kw = dict(trace=True, trace_cores=list(range(8)))
    res = run_bass_kernel_spmd(nc, in_maps, core_ids=list(range(8)), **kw)
    LAST_RESULT = res

    out = np.empty((B, S, H), dtype=np.float32)
    for c in range(8):
        b, role = c % 4, c // 4
        y = res.results[c]["y"]
        for i in range(NT):
            g = 2 * i + role
            out[b, g * QT:(g + 1) * QT, :] = y[i * QT:(i + 1) * QT, :]
    return out
